# revision 31
# baseline (speedup 1.0000x reference)
"""Trainium2 Bass kernel for nn_ConstrainModule (gnn_message_passing).

Computes, for full inputs:
    A[c,s]   = sum_{n,h,w} seg[n,c,s,h,w] * det[n,c,h,w]
    denom[c] = sum_{n,h,w} det[n,c,h,w]
    p_area   = A / denom[:,None]
    w[j]     = sum over edges (i,j) of p_area[j,i]
    probs    = det_class_probs @ w
    loss     = mean(-clip(log(probs), -100))

Sharding: data-parallel over N_obj (1024 -> 128 per core, 8 cores).

Device strategy per core (n=128 objects on the SBUF partition dim):
  - inputs are packed per det class c as raw bytes: det mask in bf16
    (1568 B) followed by the 4 seg masks in fp8e4m3 (3136 B, stochastic
    rounding on host so the quantizer is unbiased); classes are grouped
    1,2,2,2,1 per DMA chunk (singleton first for an earlier PE start,
    singleton last for a shorter compute tail); on device the regions
    are bitcast back to their dtypes.
  - TensorE contracts n: for each c, 7 accumulating matmuls with
    lhsT = det hw-chunk (112 cols, bf16) and rhs = seg (s, hw-chunk)
    (448 cols, fp8) produce psum[g, s*112+g'] cross products in fp32;
    the g==g' diagonal holds sum_n det[n,c,hw_g] * seg[n,c,s,hw_g'].
  - the diagonal is extracted on the VectorE with one fused
    scalar_tensor_tensor (mask-multiply + free-dim accumulate) per
    (c,s), writing out[g, c*4+s]; denom partials on the ScalarE via
    activation(Copy, accum_out=...) into out[:, 32+c], one per c.
  - a short burst of dummy matmuls at kernel start trips the PE HAM
    clock gate during the initial DMA wait so real matmuls run at
    2.4 GHz.
  - host sums the tiny per-core partials, applies edge weights and the
    scalar loss (a few KB of work).

Precision: stochastic rounding makes the fp8 quantizer unbiased, so the
~800K-term fp32 reductions average the ~2% per-element seg noise down
to ~3e-5 relative; det stays bf16 (~1e-5).

Self-contained: hardcodes all shapes; reads no sibling files.
"""

import numpy as np
import ml_dtypes

import concourse.bacc as bacc
import concourse.mybir as mybir
import concourse.tile as tile
from concourse.bass_utils import run_bass_kernel_spmd

N_CORES = 8
N_OBJ, C_DET, C_SEG, H, W = 1024, 8, 4, 28, 28
HW = H * W                 # 784
NS = N_OBJ // N_CORES      # 128 objects per core -> partition dim
G = 112                    # hw chunk size (lhs free dim); 784 = 7 * 112
KCH = HW // G              # 7 accumulating matmuls per class

DET_B = HW * 2             # 1568 bytes of bf16 det per class
SEG_B = C_SEG * HW         # 3136 bytes of fp8 seg per class
ROW_B = DET_B + SEG_B      # 4704 bytes per (n, c)

OUT_COLS = C_DET * C_SEG + C_DET   # 32 diag-sum cols + 8 denom cols

F32 = mybir.dt.float32
BF16 = mybir.dt.bfloat16
FP8 = mybir.dt.float8e4
NP_FP8 = ml_dtypes.float8_e4m3
U8 = mybir.dt.uint8

X_BUFS = 4
PSUM_BUFS = 4
WARMUP_MMS = 8

# classes per input tensor: x0 = class 0, x1 = classes 1..6, x2 = class 7
MID_PAIRS = 3

_program = None


def _build_program():
    nc = bacc.Bacc(
        "TRN2", target_bir_lowering=False, debug=False, num_devices=N_CORES
    )
    x0_d = nc.dram_tensor("x0", [NS, 1, ROW_B], U8, kind="ExternalInput")
    x1_d = nc.dram_tensor(
        "x1", [MID_PAIRS, NS, 2, ROW_B], U8, kind="ExternalInput"
    )
    x2_d = nc.dram_tensor("x2", [NS, 1, ROW_B], U8, kind="ExternalInput")
    mask_d = nc.dram_tensor("mask", [G, G], F32, kind="ExternalInput")
    out_d = nc.dram_tensor("out", [NS, OUT_COLS], F32, kind="ExternalOutput")

    with tile.TileContext(nc) as tc:
        with (
            tc.tile_pool(name="x", bufs=X_BUFS) as x_pool,
            tc.tile_pool(name="res", bufs=1) as res_pool,
            tc.tile_pool(name="psum", bufs=PSUM_BUFS, space="PSUM") as psum_pool,
        ):
            # PE warmup: dense dummy matmuls (zeroed operands) to flip the
            # HAM clock gate to 2.4 GHz while the first input DMA lands.
            warm_t = res_pool.tile([NS, 512], FP8)
            nc.gpsimd.memset(warm_t[:], 0.0)
            warm_ps = psum_pool.tile([8, 512], F32)
            for _ in range(WARMUP_MMS):
                nc.tensor.matmul(
                    warm_ps[:], warm_t[:, :8], warm_t[:, :512],
                    start=True, stop=True,
                )

            mask_t = res_pool.tile([G, G], F32)
            nc.scalar.dma_start(out=mask_t[:], in_=mask_d[:])
            out_t = res_pool.tile([NS, OUT_COLS], F32)
            nc.gpsimd.memset(out_t[:], 0.0)
            scratch = res_pool.tile([G, G], F32)
            act_scratch = res_pool.tile([NS, HW], BF16)

            def do_class(x_t, cj, c):
                det_v = x_t[:, cj, 0:DET_B].bitcast(BF16)           # [NS, 784]
                seg_v = x_t[:, cj, DET_B:ROW_B].bitcast(FP8).rearrange(
                    "p (s hw) -> p s hw", s=C_SEG
                )                                                   # [NS, 4, 784]
                nc.scalar.activation(
                    out=act_scratch[:],
                    in_=det_v,
                    func=mybir.ActivationFunctionType.Copy,
                    accum_out=out_t[:, 32 + c : 33 + c],
                )
                psum_t = psum_pool.tile([G, C_SEG * G], F32, tag="psum_t")
                for k in range(KCH):
                    nc.tensor.matmul(
                        psum_t[:],
                        det_v[:, k * G : (k + 1) * G],
                        seg_v[:, :, k * G : (k + 1) * G],
                        start=(k == 0),
                        stop=(k == KCH - 1),
                    )
                for s in range(C_SEG):
                    nc.vector.scalar_tensor_tensor(
                        out=scratch[:],
                        in0=psum_t[:, s * G : (s + 1) * G],
                        scalar=0.0,
                        in1=mask_t[:],
                        op0=mybir.AluOpType.bypass,
                        op1=mybir.AluOpType.mult,
                        accum_out=out_t[:G, c * C_SEG + s : c * C_SEG + s + 1],
                    )

            x_t = x_pool.tile([NS, 1, ROW_B], U8, tag="x")
            nc.sync.dma_start(out=x_t[:], in_=x0_d[:])
            do_class(x_t, 0, 0)
            for j in range(MID_PAIRS):
                x_t = x_pool.tile([NS, 2, ROW_B], U8, tag="x")
                nc.sync.dma_start(out=x_t[:], in_=x1_d[j])
                do_class(x_t, 0, 1 + 2 * j)
                do_class(x_t, 1, 2 + 2 * j)
            x_t = x_pool.tile([NS, 1, ROW_B], U8, tag="x")
            nc.sync.dma_start(out=x_t[:], in_=x2_d[:])
            do_class(x_t, 0, 7)

            nc.sync.dma_start(out=out_d[:], in_=out_t[:])

    nc.compile()
    return nc


def _get_program():
    global _program
    if _program is None:
        _program = _build_program()
    return _program


def _sr_fp8(v, rng):
    """Exact stochastic rounding to fp8e4m3: E[q(v)] = v.

    For non-negative v below fp8 max, the e4m3 bit patterns are monotone,
    so the two neighbors of v are byte-adjacent.
    """
    q0 = v.astype(NP_FP8)
    f0 = q0.astype(np.float32)
    b = q0.view(np.uint8)
    lo_b = np.where(f0 <= v, b, b - 1).astype(np.uint8)
    hi_b = lo_b + 1
    lo = lo_b.view(NP_FP8).astype(np.float32)
    hi = hi_b.view(NP_FP8).astype(np.float32)
    p = (v - lo) / (hi - lo)
    u = rng.random(v.shape, dtype=np.float32)
    out_b = np.where(u < p, hi_b, lo_b).astype(np.uint8)
    # exactly-representable values keep their encoding
    out_b = np.where(f0 == v, b, out_b)
    return out_b.view(NP_FP8)


def _pack_inputs(det_mask_probs, seg_mask_probs):
    """[N,8,28,28] f32 + [N,8,4,28,28] f32 -> per-core (x0, x1, x2) u8."""
    det = np.asarray(det_mask_probs, dtype=np.float32).reshape(
        N_CORES, NS, C_DET, HW
    )
    seg = np.asarray(seg_mask_probs, dtype=np.float32).reshape(
        N_CORES, NS, C_DET, C_SEG * HW
    )
    rng = np.random.default_rng(12345)
    det_b = det.astype(ml_dtypes.bfloat16).view(np.uint8)   # [.., C_DET, 1568]
    seg_b = _sr_fp8(seg, rng).view(np.uint8)                # [.., C_DET, 3136]
    packed = np.concatenate([det_b, seg_b], axis=3)         # [8, NS, C_DET, 4704]
    x0 = np.ascontiguousarray(packed[:, :, 0:1, :])         # [8, NS, 1, ROW_B]
    x1 = np.ascontiguousarray(
        packed[:, :, 1:7, :]
        .reshape(N_CORES, NS, MID_PAIRS, 2, ROW_B)
        .transpose(0, 2, 1, 3, 4)                           # [8, 3, NS, 2, ROW_B]
    )
    x2 = np.ascontiguousarray(packed[:, :, 7:8, :])         # [8, NS, 1, ROW_B]
    return x0, x1, x2


def _run_device(det_mask_probs, seg_mask_probs, trace=False):
    """Run the per-core reduction on all 8 cores; return (A, denom, res)."""
    nc = _get_program()
    x0, x1, x2 = _pack_inputs(det_mask_probs, seg_mask_probs)
    mask = np.eye(G, dtype=np.float32)

    in_maps = [
        {"x0": x0[r], "x1": x1[r], "x2": x2[r], "mask": mask}
        for r in range(N_CORES)
    ]
    res = run_bass_kernel_spmd(nc, in_maps, list(range(N_CORES)), trace=trace)

    A = np.zeros((C_DET, C_SEG), dtype=np.float64)
    denom = np.zeros((C_DET,), dtype=np.float64)
    for r in range(N_CORES):
        o = res.results[r]["out"]
        A += o[:G, :32].reshape(G, C_DET, C_SEG).sum(axis=0)
        denom += o[:, 32:].sum(axis=0)
    return A, denom, res


def _finish(det_class_probs, edge_i, edge_j, A, denom):
    E = np.zeros((C_DET, C_SEG), dtype=np.float64)
    np.add.at(E, (np.asarray(edge_j), np.asarray(edge_i)), 1.0)
    w = (E * (A / denom[:, None])).sum(axis=1)  # (C_DET,)
    probs = np.asarray(det_class_probs, dtype=np.float64) @ w  # (N_OBJ,)
    bce = (-np.clip(np.log(probs), -100.0, None)).mean()
    return np.asarray(bce, dtype=np.float32)


def kernel(det_class_probs, det_mask_probs, seg_mask_probs, edge_i, edge_j):
    A, denom, _ = _run_device(det_mask_probs, seg_mask_probs, trace=False)
    return _finish(det_class_probs, edge_i, edge_j, A, denom)


# revision 32
# speedup vs baseline: 1.0765x; 1.0765x over previous
"""Trainium2 Bass kernel for nn_ConstrainModule (gnn_message_passing).

Computes, for full inputs:
    A[c,s]   = sum_{n,h,w} seg[n,c,s,h,w] * det[n,c,h,w]
    denom[c] = sum_{n,h,w} det[n,c,h,w]
    p_area   = A / denom[:,None]
    w[j]     = sum over edges (i,j) of p_area[j,i]
    probs    = det_class_probs @ w
    loss     = mean(-clip(log(probs), -100))

Sharding: data-parallel over N_obj (1024 -> 128 per core, 8 cores).

Device strategy per core (n=128 objects on the SBUF partition dim):
  - inputs are packed per det class c as raw bytes: det mask in bf16
    (1568 B) followed by the 4 seg masks in fp8e4m3 (3136 B, stochastic
    rounding on host so the quantizer is unbiased), two classes per
    ~1.2MB DMA chunk; on device the regions are bitcast back.
  - TensorE contracts n: for each c, 7 accumulating matmuls with
    lhsT = det hw-chunk (112 cols, bf16) and rhs = seg (s, hw-chunk)
    (448 cols, fp8) produce psum[g, s*112+g'] cross products in fp32;
    the g==g' diagonal holds sum_n det[n,c,hw_g] * seg[n,c,s,hw_g'].
  - the diagonal is extracted on the VectorE with one fused
    scalar_tensor_tensor (mask-multiply + free-dim accumulate) per
    (c,s), writing a_all[g, c*4+s]; denom partials on the ScalarE via
    activation(Copy, accum_out=...), one per c.
  - a short burst of dummy matmuls at kernel start trips the PE HAM
    clock gate during the initial DMA wait so real matmuls run at
    2.4 GHz.
  - host sums the tiny per-core partials, applies edge weights and the
    scalar loss (a few KB of work).

Precision: stochastic rounding makes the fp8 quantizer unbiased, so the
~800K-term fp32 reductions average the ~2% per-element seg noise down
to ~3e-5 relative on A; det stays bf16 (~1e-5).

Self-contained: hardcodes all shapes; reads no sibling files.
"""

import numpy as np
import ml_dtypes

import concourse.bacc as bacc
import concourse.mybir as mybir
import concourse.tile as tile
from concourse.bass_utils import run_bass_kernel_spmd

N_CORES = 8
N_OBJ, C_DET, C_SEG, H, W = 1024, 8, 4, 28, 28
HW = H * W                 # 784
NS = N_OBJ // N_CORES      # 128 objects per core -> partition dim
G = 112                    # hw chunk size (lhs free dim); 784 = 7 * 112
KCH = HW // G              # 7 accumulating matmuls per class
CPC = 2                    # det classes per DMA chunk
NCH = C_DET // CPC         # 4 chunks

DET_B = HW * 2             # 1568 bytes of bf16 det per class
SEG_B = C_SEG * HW         # 3136 bytes of fp8 seg per class
ROW_B = DET_B + SEG_B      # 4704 bytes per (n, c)

F32 = mybir.dt.float32
BF16 = mybir.dt.bfloat16
FP8 = mybir.dt.float8e4
NP_FP8 = ml_dtypes.float8_e4m3
U8 = mybir.dt.uint8

X_BUFS = 4
PSUM_BUFS = 4
WARMUP_MMS = 8

_program = None


def _build_program():
    nc = bacc.Bacc(
        "TRN2", target_bir_lowering=False, debug=False, num_devices=N_CORES
    )
    x_d = nc.dram_tensor("x", [NCH, NS, CPC, ROW_B], U8, kind="ExternalInput")
    mask_d = nc.dram_tensor("mask", [G, G], F32, kind="ExternalInput")
    a_d = nc.dram_tensor("a", [G, C_DET * C_SEG], F32, kind="ExternalOutput")
    dsum_d = nc.dram_tensor("dsum", [NS, C_DET], F32, kind="ExternalOutput")

    with tile.TileContext(nc) as tc:
        with (
            tc.tile_pool(name="x", bufs=X_BUFS) as x_pool,
            tc.tile_pool(name="res", bufs=1) as res_pool,
            tc.tile_pool(name="psum", bufs=PSUM_BUFS, space="PSUM") as psum_pool,
        ):
            # PE warmup: dense dummy matmuls (zeroed operands) to flip the
            # HAM clock gate to 2.4 GHz while the first input DMA lands.
            warm_t = res_pool.tile([NS, 512], FP8)
            nc.gpsimd.memset(warm_t[:], 0.0)
            warm_ps = psum_pool.tile([8, 512], F32)
            for _ in range(WARMUP_MMS):
                nc.tensor.matmul(
                    warm_ps[:], warm_t[:, :8], warm_t[:, :512],
                    start=True, stop=True,
                )

            mask_t = res_pool.tile([G, G], F32)
            nc.scalar.dma_start(out=mask_t[:], in_=mask_d[:])
            a_all = res_pool.tile([G, C_DET * C_SEG], F32)
            dsum_t = res_pool.tile([NS, C_DET], F32)
            scratch = res_pool.tile([G, G], F32)
            act_scratch = res_pool.tile([NS, HW], BF16)

            for j in range(NCH):
                x_t = x_pool.tile([NS, CPC, ROW_B], U8)
                nc.sync.dma_start(out=x_t[:], in_=x_d[j])
                for cj in range(CPC):
                    c = j * CPC + cj
                    det_v = x_t[:, cj, 0:DET_B].bitcast(BF16)       # [NS, 784]
                    seg_v = x_t[:, cj, DET_B:ROW_B].bitcast(FP8).rearrange(
                        "p (s hw) -> p s hw", s=C_SEG
                    )                                               # [NS, 4, 784]
                    nc.scalar.activation(
                        out=act_scratch[:],
                        in_=det_v,
                        func=mybir.ActivationFunctionType.Copy,
                        accum_out=dsum_t[:, c : c + 1],
                    )
                    psum_t = psum_pool.tile([G, C_SEG * G], F32)
                    for k in range(KCH):
                        nc.tensor.matmul(
                            psum_t[:],
                            det_v[:, k * G : (k + 1) * G],
                            seg_v[:, :, k * G : (k + 1) * G],
                            start=(k == 0),
                            stop=(k == KCH - 1),
                        )
                    for s in range(C_SEG):
                        nc.vector.scalar_tensor_tensor(
                            out=scratch[:],
                            in0=psum_t[:, s * G : (s + 1) * G],
                            scalar=0.0,
                            in1=mask_t[:],
                            op0=mybir.AluOpType.bypass,
                            op1=mybir.AluOpType.mult,
                            accum_out=a_all[:, c * C_SEG + s : c * C_SEG + s + 1],
                        )
            nc.sync.dma_start(out=a_d[:], in_=a_all[:])
            nc.sync.dma_start(out=dsum_d[:], in_=dsum_t[:])

    nc.compile()
    return nc


def _get_program():
    global _program
    if _program is None:
        _program = _build_program()
    return _program


def _sr_fp8(v, rng):
    """Exact stochastic rounding to fp8e4m3: E[q(v)] = v.

    For non-negative v below fp8 max, the e4m3 bit patterns are monotone,
    so the two neighbors of v are byte-adjacent.
    """
    q0 = v.astype(NP_FP8)
    f0 = q0.astype(np.float32)
    b = q0.view(np.uint8)
    lo_b = np.where(f0 <= v, b, b - 1).astype(np.uint8)
    hi_b = lo_b + 1
    lo = lo_b.view(NP_FP8).astype(np.float32)
    hi = hi_b.view(NP_FP8).astype(np.float32)
    p = (v - lo) / (hi - lo)
    u = rng.random(v.shape, dtype=np.float32)
    out_b = np.where(u < p, hi_b, lo_b).astype(np.uint8)
    # exactly-representable values keep their encoding
    out_b = np.where(f0 == v, b, out_b)
    return out_b.view(NP_FP8)


def _pack_inputs(det_mask_probs, seg_mask_probs):
    """[N,8,28,28] f32 + [N,8,4,28,28] f32 -> [cores, NCH, NS, CPC, ROW_B] u8."""
    det = np.asarray(det_mask_probs, dtype=np.float32).reshape(
        N_CORES, NS, C_DET, HW
    )
    seg = np.asarray(seg_mask_probs, dtype=np.float32).reshape(
        N_CORES, NS, C_DET, C_SEG * HW
    )
    rng = np.random.default_rng(12345)
    det_b = det.astype(ml_dtypes.bfloat16).view(np.uint8)   # [.., C_DET, 1568]
    seg_b = _sr_fp8(seg, rng).view(np.uint8)                # [.., C_DET, 3136]
    packed = np.concatenate([det_b, seg_b], axis=3)         # [8, NS, C_DET, 4704]
    packed = packed.reshape(N_CORES, NS, NCH, CPC, ROW_B)
    packed = packed.transpose(0, 2, 1, 3, 4)                # [8, NCH, NS, CPC, ROW_B]
    return np.ascontiguousarray(packed)


def _run_device(det_mask_probs, seg_mask_probs, trace=False):
    """Run the per-core reduction on all 8 cores; return (A, denom, res)."""
    nc = _get_program()
    x = _pack_inputs(det_mask_probs, seg_mask_probs)
    mask = np.eye(G, dtype=np.float32)

    in_maps = [{"x": x[r], "mask": mask} for r in range(N_CORES)]
    res = run_bass_kernel_spmd(nc, in_maps, list(range(N_CORES)), trace=trace)

    A = np.zeros((C_DET, C_SEG), dtype=np.float64)
    denom = np.zeros((C_DET,), dtype=np.float64)
    for r in range(N_CORES):
        A += res.results[r]["a"].reshape(G, C_DET, C_SEG).sum(axis=0)
        denom += res.results[r]["dsum"].sum(axis=0)
    return A, denom, res


def _finish(det_class_probs, edge_i, edge_j, A, denom):
    E = np.zeros((C_DET, C_SEG), dtype=np.float64)
    np.add.at(E, (np.asarray(edge_j), np.asarray(edge_i)), 1.0)
    w = (E * (A / denom[:, None])).sum(axis=1)  # (C_DET,)
    probs = np.asarray(det_class_probs, dtype=np.float64) @ w  # (N_OBJ,)
    bce = (-np.clip(np.log(probs), -100.0, None)).mean()
    return np.asarray(bce, dtype=np.float32)


def kernel(det_class_probs, det_mask_probs, seg_mask_probs, edge_i, edge_j):
    A, denom, _ = _run_device(det_mask_probs, seg_mask_probs, trace=False)
    return _finish(det_class_probs, edge_i, edge_j, A, denom)
